# revision 23
# baseline (speedup 1.0000x reference)
"""Trainium2 Bass kernel for nn_MemoryBankV2 (memory-bank attention block).

Strategy: batch items are independent -> shard batch B=128 contiguously
across 8 NeuronCores (16 items / core), replicate the memory bank; zero
collectives.

v4: fp8 (e4m3) DoubleRow matmuls for all the heavy GEMMs (q/k/v proj,
scores, attn, Z, FFN1, FFN2) at 2x PE throughput; visibility mask folded
into the score PSUM as a bf16 matmul; bv folded past the deferred softmax
division; LN stats matmuls in bf16; item-0 rows fixed up on the host.

Elementwise chains are batched to [128, DT*R] single ops (DVE/Scalar ops
carry ~0.5us fixed overhead each; consumers contract over all DT
subtiles anyway): one consolidated [128, DT, R] PSUM tile serves the
attention accumulators, FFN2 accumulators, and gate matmuls, so the
normalize/residual/gate chains are 2-3 wide ops instead of 8-20 narrow
ones. DRAM inputs are host-swizzled to [128, K] so each dma_start is one
large 2D descriptor.

Scales: Wq/Wk/Wv/W1/W2 (+bq/bk) pre-scaled by 32 for fp8 range; 1/sqrt(D)
and 1/(32*32) folded into the Exp scale, the attention 1/32 into the Z
reciprocal, the FFN1 1/32 into the Gelu scale, the FFN2 1/32 into the
residual op. Layout: "Col" [feature-on-partitions, rows-in-free].
"""

import sys

import numpy as np

sys.path.insert(0, "/opt/trn_rl_repo")

import ml_dtypes  # noqa: E402

import concourse.bass as bass  # noqa: E402
import concourse.mybir as mybir  # noqa: E402
import concourse.tile as tile  # noqa: E402
from concourse import bacc  # noqa: E402
from concourse.bass import ds  # noqa: E402
from concourse.bass_utils import run_bass_kernel_spmd  # noqa: E402

B, T, D, L = 128, 32, 512, 2
NCORES = 8
BLOC = B // NCORES      # 16 items per core
R = BLOC * T            # 512 rows per core
M = B * T               # 4096 memory entries
DT = D // 128           # 4 feature subtiles
FT = (4 * D) // 128     # 16 ffn subtiles
MT = M // 128           # 32 memory subtiles
NCHUNK = M // 512       # 8 memory chunks (512 each)

F32 = mybir.dt.float32
BF16 = mybir.dt.bfloat16
F8 = mybir.dt.float8e4
AF = mybir.ActivationFunctionType
ALU = mybir.AluOpType
DR = mybir.MatmulPerfMode.DoubleRow
BF = ml_dtypes.bfloat16
F8NP = ml_dtypes.float8_e4m3

W8 = 32.0                       # fp8 range scale on projection weights
EXPSC = (1.0 / np.sqrt(D)) / (W8 * W8)
MASKV = -4.0e6                  # exp(EXPSC*(s+MASKV)) == 0
FFN_FP8 = True

# params tensor column layout (per layer l, base = 48*l)
P_BQ, P_BK, P_BV, P_G1, P_BE1, P_B1, P_B2, P_G2, P_BE2 = 0, 4, 8, 12, 16, 20, 36, 40, 44
P_BS = 96
P_COLS = 100


def _bc(ap, n, r=R):
    """broadcast [128, r] -> [128, n, r] (stride-0 middle dim)."""
    return ap.unsqueeze(1).broadcast_to([128, n, r])


def _layernorm(nc, psum, tmps, x, prm, gcol, bcol, out_bf, onesi, epsln,
               drain_filler=lambda n: None, write_f32=True):
    """LN over the partition axis of x ([128, DT, R] fp32), in place;
    also writes out_bf (gain/bias folded into per-partition scale/bias
    ops). Stats matmuls run in bf16; elementwise ops batched."""
    xst = tmps.tile([128, DT, R], BF16, tag="xst", bufs=1, name="ln_xst")
    nc.vector.tensor_copy(xst, x)
    mups = psum.tile([128, R], F32, tag="mm", bufs=3, name="ln_mu")
    for a in range(DT):
        nc.tensor.matmul(mups, onesi, xst[:, a, :], start=(a == 0), stop=(a == DT - 1))
    drain_filler(3)
    sq = tmps.tile([128, DT, R], BF16, tag="sq", bufs=1, name="ln_sqt")
    nc.vector.tensor_mul(sq, xst, xst)
    sqps = psum.tile([128, R], F32, tag="mm", bufs=3, name="ln_sq")
    for a in range(DT):
        nc.tensor.matmul(sqps, onesi, sq[:, a, :], start=(a == 0), stop=(a == DT - 1))
    drain_filler(9)
    musb = tmps.tile([128, R], F32, tag="lns", bufs=2, name="ln_musb")
    nc.vector.tensor_copy(musb, mups)
    mu2 = tmps.tile([128, R], F32, tag="lns", bufs=2, name="ln_mu2")
    nc.vector.tensor_mul(mu2, musb, musb)
    var = tmps.tile([128, R], F32, tag="lns", bufs=2, name="ln_var")
    nc.vector.tensor_sub(var, sqps, mu2)
    sd = tmps.tile([128, R], F32, tag="lns", bufs=2, name="ln_sd")
    nc.scalar.activation(out=sd, in_=var, func=AF.Sqrt, bias=epsln, scale=1.0)
    rstd = tmps.tile([128, R], F32, tag="lns", bufs=2, name="ln_rstd")
    nc.vector.reciprocal_approx_fast(rstd, sd)
    xm = tmps.tile([128, DT, R], F32, tag="lnxm", bufs=1, name="ln_xm")
    nc.vector.tensor_sub(xm, x, _bc(mups, DT))
    t = xm
    nc.vector.tensor_mul(t, xm, _bc(rstd, DT))
    for a in range(DT):
        # bf16/fp8 copy first: it unblocks the next matmul consumer
        nc.scalar.activation(out=out_bf[:, a, :], in_=t[:, a, :], func=AF.Identity,
                             bias=prm[:, bcol + a:bcol + a + 1],
                             scale=prm[:, gcol:gcol + 1])
        if write_f32:
            nc.vector.tensor_scalar(out=x[:, a, :], in0=t[:, a, :],
                                    scalar1=prm[:, gcol:gcol + 1],
                                    scalar2=prm[:, bcol + a:bcol + a + 1],
                                    op0=ALU.mult, op1=ALU.add)


def _build():
    nc = bacc.Bacc("TRN2", target_bir_lowering=False, debug=False)

    # all host-swizzled to [128, K] (partition-major, one 2D DMA each)
    memT_d = nc.dram_tensor("memT8", [128, DT * M], F8, kind="ExternalInput").ap()
    xT0_d = nc.dram_tensor("xT0", [128, DT * R], F32, kind="ExternalInput").ap()
    xT0bf_d = nc.dram_tensor("xT0bf", [128, DT * R], BF16, kind="ExternalInput").ap()
    xT08_d = nc.dram_tensor("xT08", [128, DT * R], F8, kind="ExternalInput").ap()
    thr_d = nc.dram_tensor("thr", [128, R], BF16, kind="ExternalInput").ap()
    ind_d = nc.dram_tensor("ind", [128, MT * 128], BF16, kind="ExternalInput").ap()
    prm_d = nc.dram_tensor("params", [128, P_COLS], F32, kind="ExternalInput").ap()
    WDT = F8 if FFN_FP8 else BF16
    wq_d, wk_d, wv_d, w1_d, w2_d = [], [], [], [], []
    for l in range(L):
        # wq j-major so qproj j=0 can start on a 64KB DMA
        wq_d.append(nc.dram_tensor(f"wq8{l}", [128, DT * D], F8, kind="ExternalInput").ap())
        wk_d.append(nc.dram_tensor(f"wk8{l}", [128, DT * D], F8, kind="ExternalInput").ap())
        wv_d.append(nc.dram_tensor(f"wv8{l}", [128, DT * D], F8, kind="ExternalInput").ap())
        # w1 chunk-major: [(og a) col] with 4 chunks, w2 plain: [(og s) col]
        w1_d.append(nc.dram_tensor(f"w1{l}", [128, 4 * DT * 512], WDT, kind="ExternalInput").ap())
        w2_d.append(nc.dram_tensor(f"w2{l}", [128, FT * D], WDT, kind="ExternalInput").ap())
    ws_d = nc.dram_tensor("ws", [128, 8 * D], BF16, kind="ExternalInput").ap()
    out_d = nc.dram_tensor("outT", [D, R], F32, kind="ExternalOutput").ap()

    with tile.TileContext(nc) as tc:
        with (
            tc.tile_pool(name="sb", bufs=1) as sb,
            tc.tile_pool(name="ps", bufs=1, space="PSUM") as ps,
        ):
            # --- resident inputs; DMA issue order = need order ------------------
            x08 = sb.tile([128, DT, R], F8, tag="x08", name="x08_sb")
            for a in range(DT):
                nc.sync.dma_start(out=x08[:, a, :], in_=xT08_d[:, ds(a * R, R)])
            prm = sb.tile([128, P_COLS], F32, tag="prm", name="prm_sb")
            nc.sync.dma_start(out=prm, in_=prm_d[:, :])

            def load_layer_weights(l):
                # wq j-major layout: [128, j, a, 128]
                wq = sb.tile([128, DT, DT, 128], F8, tag="wq", bufs=2, name="wq_sb")
                for j in range(DT):
                    nc.sync.dma_start(
                        out=wq[:, j, :, :],
                        in_=wq_d[l][:, ds(j * DT * 128, DT * 128)].rearrange(
                            "p (a n) -> p a n", a=DT))
                wk = sb.tile([128, DT, D], F8, tag="wk", bufs=2, name="wk_sb")
                wv = sb.tile([128, DT, D], F8, tag="wv", bufs=2, name="wv_sb")
                nc.sync.dma_start(out=wk, in_=wk_d[l].rearrange("p (a n) -> p a n", a=DT))
                nc.sync.dma_start(out=wv, in_=wv_d[l].rearrange("p (a n) -> p a n", a=DT))
                return wq, wk, wv

            layer_w = [load_layer_weights(0)]
            memT = sb.tile([128, DT, M], F8, tag="memT", name="memT_sb")
            for c0, c1 in ((0, 1024), (1024, 2048), (2048, M)):
                for a in range(DT):
                    nc.sync.dma_start(out=memT[:, a, c0:c1],
                                      in_=memT_d[:, a * M + c0:a * M + c1])
            x0bf = sb.tile([128, DT, R], BF16, tag="x0bf", name="x0bf_sb")
            nc.sync.dma_start(out=x0bf, in_=xT0bf_d.rearrange("p (a n) -> p a n", a=DT))
            x = sb.tile([128, DT, R], F32, tag="x", name="x_sb")
            x0 = sb.tile([128, DT, R], F32, tag="x0", name="x0_sb")
            nc.sync.dma_start(out=x0, in_=xT0_d.rearrange("p (a n) -> p a n", a=DT))
            # x starts as a copy of x0 (gpsimd: off the critical engines)
            for a in range(DT):
                nc.gpsimd.tensor_copy(x[:, a, :], x0[:, a, :])
            thr = sb.tile([128, R], BF16, tag="thr", name="thr_sb")
            nc.sync.dma_start(out=thr, in_=thr_d[:, :])
            indb = sb.tile([128, MT, 128], BF16, tag="ind", name="ind_sb")
            nc.sync.dma_start(out=indb, in_=ind_d.rearrange("p (t m) -> p t m", m=128))
            ones8 = sb.tile([128, 2, 128], F8, tag="ones8", name="ones8_sb")
            nc.vector.memset(ones8, 1.0)
            onesi = sb.tile([128, 128], BF16, tag="onesi", name="onesi_sb")
            nc.vector.memset(onesi, 1.0 / D)
            epsln = sb.tile([128, 1], F32, tag="epsln", name="epsln_sb")
            nc.vector.memset(epsln, 1e-5)

            # kproj group: kT8[:, j, c*512:+512] = DoubleRow(wk, memT) + bk
            def emit_kproj_group(l, wk, kT, c, j, tag="mm", tagbufs=3,
                                 cast_on_scalar=False):
                base = 48 * l
                kps = ps.tile([128, 512], F32, tag=tag, bufs=tagbufs, name="k_ps")
                for a2 in range(DT // 2):
                    nc.tensor.matmul(kps, wk[:, ds(2 * a2, 2), ds(j * 128, 128)],
                                     memT[:, ds(2 * a2, 2), ds(c * 512, 512)],
                                     start=(a2 == 0), stop=(a2 == DT // 2 - 1),
                                     perf_mode=DR)
                dst = kT[:, j, ds(c * 512, 512)]
                bias_ap = prm[:, base + P_BK + j:base + P_BK + j + 1]
                if cast_on_scalar:
                    nc.scalar.activation(out=dst, in_=kps, func=AF.Identity,
                                         bias=bias_ap, scale=1.0)
                else:
                    nc.vector.tensor_scalar(out=dst, in0=kps, scalar1=bias_ap,
                                            scalar2=None, op0=ALU.add)

            # filler queue: pending PE work for the next layer, drained into
            # the PE-idle zones (LN chains / FFN) of the current layer
            filler: list = []

            def drain_filler(n):
                for _ in range(min(n, len(filler))):
                    filler.pop(0)()

            # per-layer per-feature bias broadcast tiles (bv, b2), built
            # off the critical path on Scalar
            def make_bias_bcast(col, name):
                bt = sb.tile([128, DT, 1], F32, tag="bt", bufs=2, name=name)
                for j in range(DT):
                    nc.scalar.activation(out=bt[:, j, :], in_=epsln,
                                         func=AF.Identity, scale=0.0,
                                         bias=prm[:, col + j:col + j + 1])
                return bt.broadcast_to([128, DT, R])

            x8cur = x08  # fp8 attention input of the current layer
            vpre_cur = []
            for l in range(L):
                base = 48 * l
                wq, wk, wv = layer_w[l]

                # --- q projection (fp8 DR matmuls, fp8 output) ------------------
                q8 = sb.tile([128, DT, R], F8, tag="q8", bufs=1, name="q_sb")
                for j in range(DT):
                    qps = ps.tile([128, R], F32, tag="mm", bufs=3, name="q_ps")
                    for a2 in range(DT // 2):
                        nc.tensor.matmul(qps, wq[:, j, ds(2 * a2, 2), :],
                                         x8cur[:, ds(2 * a2, 2), :],
                                         start=(a2 == 0), stop=(a2 == DT // 2 - 1),
                                         perf_mode=DR)
                    nc.vector.tensor_scalar(out=q8[:, j, :], in0=qps,
                                            scalar1=prm[:, base + P_BQ + j:base + P_BQ + j + 1],
                                            scalar2=None, op0=ALU.add)

                # --- kT projection ([dout, m] fp8, resident) --------------------
                if l == 0:
                    kT = sb.tile([128, DT, M], F8, tag="kT", name="kT_sb")
                    for c in range(NCHUNK):
                        for j in range(DT):
                            emit_kproj_group(0, wk, kT, c, j)
                else:
                    kT = kT_next  # noqa: F821  (built by layer l-1's filler)
                    drain_filler(len(filler))  # any leftovers

                bvt = make_bias_bcast(base + P_BV, "bvt_sb")

                # --- attention (mt pairs, DoubleRow fp8) ------------------------
                # one big accumulator tile: [128, j, R] = attnT
                bigps = ps.tile([128, DT, R], F32, tag="attn", bufs=1, name="attn_ps")
                zps = ps.tile([128, R], F32, tag="z", bufs=1, name="z_ps")
                for mp in range(MT // 2):
                    if mp < len(vpre_cur):
                        v2 = vpre_cur[mp]  # prefetched during previous layer
                    else:
                        v2 = sb.tile([128, 2, D], F8, tag="vsb", bufs=3, name="v_sb")
                        for i in range(2):
                            mt = 2 * mp + i
                            vps = ps.tile([128, D], F32, tag="mm", bufs=3, name="v_ps")
                            for a2 in range(DT // 2):
                                nc.tensor.matmul(vps, memT[:, ds(2 * a2, 2), ds(mt * 128, 128)],
                                                 wv[:, ds(2 * a2, 2), :],
                                                 start=(a2 == 0), stop=(a2 == DT // 2 - 1),
                                                 perf_mode=DR)
                            if i == 0:
                                nc.vector.tensor_copy(v2[:, i, :], vps)
                            else:
                                nc.scalar.activation(out=v2[:, i, :], in_=vps,
                                                     func=AF.Copy)
                    # scores + mask -> masked exp, per half of the pair
                    e2 = sb.tile([128, 2, R], F8, tag="e", bufs=3, name="e_sb")
                    for i in range(2):
                        mt = 2 * mp + i
                        sps = ps.tile([128, R], F32, tag="mm", bufs=3, name="s_ps")
                        for a2 in range(DT // 2):
                            nc.tensor.matmul(sps, kT[:, ds(2 * a2, 2), ds(mt * 128, 128)],
                                             q8[:, ds(2 * a2, 2), :],
                                             start=(a2 == 0), stop=False,
                                             perf_mode=DR, skip_group_check=True)
                        nc.tensor.matmul(sps, indb[:, mt, :], thr,
                                         start=False, stop=True,
                                         skip_group_check=True)
                        nc.scalar.activation(out=e2[:, i, :], in_=sps,
                                             func=AF.Exp, scale=EXPSC)
                    nc.tensor.matmul(zps, ones8, e2, start=(mp == 0),
                                     stop=(mp == MT // 2 - 1),
                                     perf_mode=DR, skip_group_check=True)
                    for j in range(DT):
                        nc.tensor.matmul(bigps[:, j, :], v2[:, :, ds(j * 128, 128)], e2,
                                         start=(mp == 0), stop=(mp == MT // 2 - 1),
                                         perf_mode=DR, skip_group_check=True)

                # enqueue next layer's kT projection as PE filler work
                vpre_next = []
                if l + 1 < L:
                    layer_w.append(load_layer_weights(l + 1))
                    kT_next = sb.tile([128, DT, M], F8, tag="kT", name="kTn_sb")
                    wk_next = layer_w[l + 1][1]
                    for c in range(NCHUNK):
                        for j in range(DT):
                            tg, tb = ("z", 1) if (c * DT + j) % 2 else ("mm", 3)
                            filler.append(
                                lambda ll=l + 1, c=c, j=j, wkn=wk_next,
                                ktn=kT_next, tg=tg, tb=tb:
                                emit_kproj_group(ll, wkn, ktn, c, j, tg, tb,
                                                 cast_on_scalar=(c + j) % 2 == 0))
                    wv_next = layer_w[l + 1][2]

                    def emit_vpre(mp_, vt):
                        for i in range(2):
                            mt = 2 * mp_ + i
                            tg, tb = ("z", 1) if i else ("mm", 3)
                            vp = ps.tile([128, D], F32, tag=tg, bufs=tb, name="vp_ps")
                            for a2 in range(DT // 2):
                                nc.tensor.matmul(vp, memT[:, ds(2 * a2, 2), ds(mt * 128, 128)],
                                                 wv_next[:, ds(2 * a2, 2), :],
                                                 start=(a2 == 0), stop=(a2 == DT // 2 - 1),
                                                 perf_mode=DR)
                            if i == 0:
                                nc.vector.tensor_copy(vt[:, i, :], vp)
                            else:
                                nc.scalar.activation(out=vt[:, i, :], in_=vp,
                                                     func=AF.Copy)

                    for mp_ in range(8):
                        vt = sb.tile([128, 2, D], F8, tag="vpre", bufs=8,
                                     name="vpre_sb")
                        vpre_next.append(vt)
                        filler.append(lambda mp_=mp_, vt=vt: emit_vpre(mp_, vt))

                # normalize + residual into x, batched:
                # x += bigps * rz + bvt   (1/32 of the v-scale inside rz)
                zt = sb.tile([128, R], F32, tag="at", bufs=2, name="zt_sb")
                nc.scalar.activation(out=zt, in_=zps, func=AF.Copy, bias=1e-9,
                                     scale=float(W8))
                rz = sb.tile([128, R], F32, tag="rz", bufs=1, name="rz_sb")
                nc.vector.reciprocal_approx_fast(rz, zt)
                at = sb.tile([128, DT, R], F32, tag="at2", bufs=1, name="at_sb")
                nc.vector.tensor_mul(at, bigps, _bc(rz, DT))
                at2 = sb.tile([128, DT, R], F32, tag="at3", bufs=1, name="at2_sb")
                nc.vector.tensor_add(at2, at, bvt)
                nc.vector.tensor_add(x, x, at2)
                drain_filler(4)

                # last layer: the gate's x0 half becomes filler work
                if l == L - 1:
                    ws0c = sb.tile([128, DT, 512], BF16, tag="wsc", bufs=2,
                                   name="ws0c_sb")
                    nc.sync.dma_start(
                        out=ws0c,
                        in_=ws_d[:, 0:DT * 512].rearrange("p (s n) -> p s n", s=DT))
                    gstash = sb.tile([128, DT, R], BF16, tag="gstash", bufs=1,
                                     name="gstash_sb")

                    def emit_gate_x0(j):
                        gxp = ps.tile([128, R], F32, tag="z", bufs=1, name="gx_ps")
                        for c in range(DT):
                            nc.tensor.matmul(gxp, ws0c[:, c, ds(j * 128, 128)],
                                             x0bf[:, c, :],
                                             start=(c == 0), stop=(c == DT - 1))
                        nc.vector.tensor_scalar(out=gstash[:, j, :], in0=gxp,
                                                scalar1=prm[:, P_BS + j:P_BS + j + 1],
                                                scalar2=None, op0=ALU.add)

                    for j in range(DT):
                        filler.append(lambda j=j: emit_gate_x0(j))

                # LN1 (in place); fp8 copy feeds the (fp8) FFN1
                xln8 = sb.tile([128, DT, R], F8 if FFN_FP8 else BF16,
                               tag="xbf", bufs=2, name="xln8_sb")
                _layernorm(nc, ps, sb, x, prm, base + P_G1, base + P_BE1, xln8,
                           onesi, epsln, drain_filler)

                b2t = make_bias_bcast(base + P_B2, "b2t_sb")

                # FFN1 -> FFN2 fused over the 4D dim (fp8 DR)
                f2big = ps.tile([128, DT, R], F32, tag="attn", bufs=1, name="f2_ps")

                w1c, w2c = {}, {}

                def load_ffn_chunk(og):
                    w1c[og] = sb.tile([128, DT, 512], WDT, tag="wc", bufs=4,
                                      name="w1c_sb")
                    nc.sync.dma_start(
                        out=w1c[og],
                        in_=w1_d[l][:, ds(og * DT * 512, DT * 512)].rearrange(
                            "p (a n) -> p a n", a=DT))
                    w2c[og] = sb.tile([128, DT, 512], WDT, tag="wc", bufs=4,
                                      name="w2c_sb")
                    nc.sync.dma_start(
                        out=w2c[og],
                        in_=w2_d[l][:, ds(og * DT * D, DT * D)].rearrange(
                            "p (s n) -> p s n", s=DT))

                load_ffn_chunk(0)

                def emit_f2(h2, op):
                    # h2: [128, 2, R] fp8 pair of FFN1 outputs (o=2*op, 2*op+1)
                    o0 = 2 * op
                    for j in range(DT):
                        nc.tensor.matmul(
                            f2big[:, j, :],
                            w2c[o0 // 4][:, ds(o0 % 4, 2), ds(j * 128, 128)], h2,
                            start=(op == 0), stop=(op == FT // 2 - 1),
                            perf_mode=DR, skip_group_check=True)

                hq = []
                h2 = None
                for o in range(FT):
                    fps = ps.tile([128, R], F32, tag="mm", bufs=3, name="f1_ps")
                    for a2 in range(DT // 2):
                        nc.tensor.matmul(fps,
                                         w1c[o // 4][:, ds(2 * a2, 2), ds((o % 4) * 128, 128)],
                                         xln8[:, ds(2 * a2, 2), :],
                                         start=(a2 == 0), stop=(a2 == DT // 2 - 1),
                                         perf_mode=DR)
                    if o % 2 == 0:
                        h2 = sb.tile([128, 2, R], F8, tag="h", bufs=4, name="h_sb")
                    nc.scalar.activation(out=h2[:, o % 2, :], in_=fps, func=AF.Gelu,
                                         bias=prm[:, base + P_B1 + o:base + P_B1 + o + 1],
                                         scale=(1.0 / W8 if FFN_FP8 else 1.0))
                    if o % 2 == 1:
                        hq.append((h2, o // 2))
                        if len(hq) > 1:
                            emit_f2(*hq.pop(0))
                    drain_filler(1)
                    if o % 4 == 3 and o // 4 + 1 < 4:
                        load_ffn_chunk(o // 4 + 1)
                for h_o in hq:
                    emit_f2(*h_o)
                # x += f2big/32 + b2, batched (scale+bias fused in one stt)
                f2b = sb.tile([128, DT, R], F32, tag="at3", bufs=1, name="f2b_sb")
                nc.vector.scalar_tensor_tensor(
                    out=f2b, in0=f2big, scalar=(1.0 / W8 if FFN_FP8 else 1.0),
                    in1=b2t, op0=ALU.mult, op1=ALU.add)
                nc.vector.tensor_add(x, x, f2b)

                if l == L - 1:
                    ws1c = sb.tile([128, DT, 512], BF16, tag="wsc", bufs=2,
                                   name="ws1c_sb")
                    nc.sync.dma_start(
                        out=ws1c,
                        in_=ws_d[:, DT * 512:8 * 512].rearrange("p (s n) -> p s n", s=DT))

                if l + 1 < L:
                    # LN2 (in place) + bf16 copy for next layer
                    xbf = sb.tile([128, DT, R], BF16, tag="xbf", bufs=2, name="xbf_sb")
                    _layernorm(nc, ps, sb, x, prm, base + P_G2, base + P_BE2, xbf,
                               onesi, epsln, drain_filler)
                    # fp8 copy for next layer's qproj
                    x8n = sb.tile([128, DT, R], F8, tag="x8n", bufs=1, name="x8n_sb")
                    nc.vector.tensor_copy(x8n, xbf)
                    x8cur = x8n
                else:
                    # final LN2 + gate + output, split in row chunks so
                    # later chunks' PE work overlaps earlier chunks' DVE/
                    # Scalar chains and out DMAs start early; the last
                    # chunk is narrow to shorten the closing serial chain.
                    # Scalar runs only Sqrt/Sigmoid here (xbf writes go to
                    # DVE) to avoid activation-table reloads mid-chain.
                    # out = xbf + g*(x0 - xbf); item-0 rows fixed on host.
                    xbf = sb.tile([128, DT, R], BF16, tag="xbf", bufs=2,
                                  name="xbf_sb")
                    gps = ps.tile([128, DT, R], F32, tag="attn", bufs=1,
                                  name="ga_ps")
                    for r0, rw in ((0, 256), (256, 128), (384, 128)):
                        rsl = ds(r0, rw)
                        xst = sb.tile([128, DT, rw], BF16, tag="xst2", bufs=2,
                                      name="fl_xst")
                        nc.vector.tensor_copy(xst, x[:, :, rsl])
                        mups = ps.tile([128, rw], F32, tag="mm", bufs=3, name="fl_mu")
                        for a in range(DT):
                            nc.tensor.matmul(mups, onesi, xst[:, a, :],
                                             start=(a == 0), stop=(a == DT - 1))
                        sq = sb.tile([128, DT, rw], BF16, tag="sq2", bufs=2,
                                     name="fl_sq")
                        nc.vector.tensor_mul(sq, xst, xst)
                        sqps = ps.tile([128, rw], F32, tag="mm", bufs=3, name="fl_sqp")
                        for a in range(DT):
                            nc.tensor.matmul(sqps, onesi, sq[:, a, :],
                                             start=(a == 0), stop=(a == DT - 1))
                        musb = sb.tile([128, rw], F32, tag="lns2", bufs=2, name="fl_musb")
                        nc.vector.tensor_copy(musb, mups)
                        mu2 = sb.tile([128, rw], F32, tag="lns2", bufs=2, name="fl_mu2")
                        nc.vector.tensor_mul(mu2, musb, musb)
                        var = sb.tile([128, rw], F32, tag="lns2", bufs=2, name="fl_var")
                        nc.vector.tensor_sub(var, sqps, mu2)
                        sd = sb.tile([128, rw], F32, tag="lns2", bufs=2, name="fl_sd")
                        nc.scalar.activation(out=sd, in_=var, func=AF.Sqrt,
                                             bias=epsln, scale=1.0)
                        rstd = sb.tile([128, rw], F32, tag="lns2", bufs=2, name="fl_rstd")
                        nc.vector.reciprocal_approx_fast(rstd, sd)
                        xm = sb.tile([128, DT, rw], F32, tag="lnxm2", bufs=2, name="fl_xm")
                        nc.vector.tensor_sub(xm, x[:, :, rsl], _bc(mups, DT, rw))
                        t = xm
                        nc.vector.tensor_mul(t, xm, _bc(rstd, DT, rw))
                        for a in range(DT):
                            nc.vector.tensor_scalar(
                                out=xbf[:, a, rsl], in0=t[:, a, :],
                                scalar1=prm[:, base + P_G2:base + P_G2 + 1],
                                scalar2=prm[:, base + P_BE2 + a:base + P_BE2 + a + 1],
                                op0=ALU.mult, op1=ALU.add)
                        for j in range(DT):
                            for c in range(DT):
                                nc.tensor.matmul(gps[:, j, rsl],
                                                 ws1c[:, c, ds(j * 128, 128)],
                                                 xbf[:, c, rsl],
                                                 start=(c == 0), stop=(c == DT - 1),
                                                 skip_group_check=True)
                        dxh = sb.tile([128, DT, rw], F32, tag="dxh", bufs=2,
                                      name="fl_dx")
                        nc.vector.tensor_sub(dxh, x0[:, :, rsl], xbf[:, :, rsl])
                        tgh = sb.tile([128, DT, rw], F32, tag="gt", bufs=3,
                                      name="fl_tg")
                        nc.vector.tensor_add(tgh, gps[:, :, rsl], gstash[:, :, rsl])
                        gh = sb.tile([128, DT, rw], F32, tag="gt", bufs=3,
                                     name="fl_g")
                        nc.scalar.activation(out=gh, in_=tgh, func=AF.Sigmoid)
                        m2h = sb.tile([128, DT, rw], F32, tag="gt", bufs=3,
                                      name="fl_m2")
                        nc.vector.tensor_mul(m2h, gh, dxh)
                        ovh = sb.tile([128, DT, rw], F32, tag="gt", bufs=3,
                                      name="fl_ov")
                        nc.vector.tensor_add(ovh, xbf[:, :, rsl], m2h)
                        for j in range(DT):
                            nc.sync.dma_start(
                                out=out_d[j * 128:(j + 1) * 128, r0:r0 + rw],
                                in_=ovh[:, j, :])
                vpre_cur = vpre_next

    nc.compile()
    return nc


_NC = None


def _get_nc():
    global _NC
    if _NC is None:
        _NC = _build()
    return _NC


def _sw(x, p=128):
    """[A*p, N] -> [p, A*N] partition-major swizzle (row pp = concat_a x[a*p+pp])."""
    a = x.shape[0] // p
    return np.ascontiguousarray(
        x.reshape(a, p, -1).transpose(1, 0, 2).reshape(p, -1))


def _make_in_maps(inputs):
    cog = np.asarray(inputs["cognition_features"], np.float32)
    flat = cog.reshape(M, D)
    cogT = np.ascontiguousarray(flat.T)          # [D, M] fp32
    wdt = F8NP if FFN_FP8 else BF
    wsc = W8 if FFN_FP8 else 1.0

    common = {"memT8": _sw(cogT).astype(F8NP)}
    for l in range(L):
        # wq j-major: [p, j, a, 128] from (32*Wq).T [D, D]
        wqt = np.asarray(inputs["Wq"][l], np.float32).T * W8
        common[f"wq8{l}"] = np.ascontiguousarray(
            wqt.reshape(DT, 128, DT, 128).transpose(1, 2, 0, 3).reshape(128, -1)
        ).astype(F8NP)
        common[f"wk8{l}"] = _sw(
            np.asarray(inputs["Wk"][l], np.float32).T * W8).astype(F8NP)
        common[f"wv8{l}"] = _sw(
            np.asarray(inputs["Wv"][l], np.float32).T * W8).astype(F8NP)
        # w1 chunk-major: [p, (og a) col] from (wsc*W1).T [D, 4D]
        w1t = np.asarray(inputs["W1"][l], np.float32).T * wsc
        common[f"w1{l}"] = np.ascontiguousarray(
            w1t.reshape(DT, 128, 4, 512).transpose(1, 2, 0, 3).reshape(128, -1)
        ).astype(wdt)
        common[f"w2{l}"] = _sw(
            np.asarray(inputs["W2"][l], np.float32).T * wsc).astype(wdt)
    common["ws"] = _sw(np.asarray(inputs["Ws"], np.float32).T).astype(BF)

    prm = np.zeros((128, P_COLS), np.float32)

    def put(col, vec):
        v = np.asarray(vec, np.float32).reshape(-1, 128)
        for j in range(v.shape[0]):
            prm[:, col + j] = v[j]

    for l in range(L):
        base = 48 * l
        put(base + P_BQ, np.asarray(inputs["bq"][l], np.float32) * W8)
        put(base + P_BK, np.asarray(inputs["bk"][l], np.float32) * W8)
        put(base + P_BV, inputs["bv"][l])
        put(base + P_G1, inputs["ln1_g"][l])
        put(base + P_BE1, inputs["ln1_b"][l])
        put(base + P_B1, inputs["b1"][l])
        put(base + P_B2, inputs["b2"][l])
        put(base + P_G2, inputs["ln2_g"][l])
        put(base + P_BE2, inputs["ln2_b"][l])
    put(P_BS, inputs["bs"])
    common["params"] = prm

    # ind[theta, mt*128+p] = 1 iff theta == item(mt*128+p)  (mask rank factor)
    item_of_m = np.arange(M) // T
    ind = np.zeros((128, MT * 128), np.float32)
    ind[item_of_m, np.arange(M)] = 1.0
    common["ind"] = ind.astype(BF)

    in_maps = []
    for d in range(NCORES):
        rows = slice(d * R, (d + 1) * R)
        b_of_r = np.arange(d * R, (d + 1) * R) // T
        im = dict(common)
        xt = np.ascontiguousarray(cogT[:, rows])
        im["xT0"] = _sw(xt)
        im["xT0bf"] = _sw(xt.astype(BF))
        im["xT08"] = _sw(xt.astype(F8NP))
        # thr[theta, r] = MASKV iff theta >= item(r) (else 0)
        th = np.where(np.arange(128)[:, None] >= b_of_r[None, :], MASKV, 0.0)
        im["thr"] = th.astype(BF)
        in_maps.append(im)
    return in_maps


def _run(in_maps, trace=False):
    nc = _get_nc()
    return run_bass_kernel_spmd(nc, in_maps, list(range(NCORES)), trace=trace)


def kernel(**inputs):
    in_maps = _make_in_maps(inputs)
    res = _run(in_maps)
    outT = np.empty((M, D), np.float32)
    for d in range(NCORES):
        outT[d * R:(d + 1) * R, :] = res.results[d]["outT"].T
    out = outT.reshape(B, T, D)
    # item 0 attends over an empty bank: out == input there
    out[0] = np.asarray(inputs["cognition_features"], np.float32)[0]
    return out


if __name__ == "__main__":
    _build()
    print("build ok")


# revision 24
# speedup vs baseline: 1.0296x; 1.0296x over previous
"""Trainium2 Bass kernel for nn_MemoryBankV2 (memory-bank attention block).

Strategy: batch items are independent -> shard batch B=128 contiguously
across 8 NeuronCores (16 items / core), replicate the memory bank; zero
collectives.

v4: fp8 (e4m3) DoubleRow matmuls for all the heavy GEMMs (q/k/v proj,
scores, attn, Z, FFN1, FFN2) at 2x PE throughput; visibility mask folded
into the score PSUM as a bf16 matmul; bv folded past the deferred softmax
division; LN stats matmuls in bf16; item-0 rows fixed up on the host.

Elementwise chains are batched to [128, DT*R] single ops (DVE/Scalar ops
carry ~0.5us fixed overhead each; consumers contract over all DT
subtiles anyway): one consolidated [128, DT, R] PSUM tile serves the
attention accumulators, FFN2 accumulators, and gate matmuls, so the
normalize/residual/gate chains are 2-3 wide ops instead of 8-20 narrow
ones. DRAM inputs are host-swizzled to [128, K] so each dma_start is one
large 2D descriptor.

Scales: Wq/Wk/Wv/W1/W2 (+bq/bk) pre-scaled by 32 for fp8 range; 1/sqrt(D)
and 1/(32*32) folded into the Exp scale, the attention 1/32 into the Z
reciprocal, the FFN1 1/32 into the Gelu scale, the FFN2 1/32 into the
residual op. Layout: "Col" [feature-on-partitions, rows-in-free].
"""

import sys

import numpy as np

sys.path.insert(0, "/opt/trn_rl_repo")

import ml_dtypes  # noqa: E402

import concourse.bass as bass  # noqa: E402
import concourse.mybir as mybir  # noqa: E402
import concourse.tile as tile  # noqa: E402
from concourse import bacc  # noqa: E402
from concourse.bass import ds  # noqa: E402
from concourse.bass_utils import run_bass_kernel_spmd  # noqa: E402

B, T, D, L = 128, 32, 512, 2
NCORES = 8
BLOC = B // NCORES      # 16 items per core
R = BLOC * T            # 512 rows per core
M = B * T               # 4096 memory entries
DT = D // 128           # 4 feature subtiles
FT = (4 * D) // 128     # 16 ffn subtiles
MT = M // 128           # 32 memory subtiles
NCHUNK = M // 512       # 8 memory chunks (512 each)

F32 = mybir.dt.float32
BF16 = mybir.dt.bfloat16
F8 = mybir.dt.float8e4
AF = mybir.ActivationFunctionType
ALU = mybir.AluOpType
DR = mybir.MatmulPerfMode.DoubleRow
BF = ml_dtypes.bfloat16
F8NP = ml_dtypes.float8_e4m3

W8 = 32.0                       # fp8 range scale on projection weights
EXPSC = (1.0 / np.sqrt(D)) / (W8 * W8)
MASKV = -4.0e6                  # exp(EXPSC*(s+MASKV)) == 0
FFN_FP8 = True

# params tensor column layout (per layer l, base = 48*l)
P_BQ, P_BK, P_BV, P_G1, P_BE1, P_B1, P_B2, P_G2, P_BE2 = 0, 4, 8, 12, 16, 20, 36, 40, 44
P_BS = 96
P_COLS = 100


def _bc(ap, n, r=R):
    """broadcast [128, r] -> [128, n, r] (stride-0 middle dim)."""
    return ap.unsqueeze(1).broadcast_to([128, n, r])


def _layernorm(nc, psum, tmps, x, prm, gcol, bcol, out_bf, onesi, epsln,
               drain_filler=lambda n: None, write_f32=True):
    """LN over the partition axis of x ([128, DT, R] fp32), in place;
    also writes out_bf (gain/bias folded into per-partition scale/bias
    ops). Stats matmuls run in bf16; elementwise ops batched."""
    xst = tmps.tile([128, DT, R], BF16, tag="xst", bufs=1, name="ln_xst")
    nc.vector.tensor_copy(xst, x)
    mups = psum.tile([128, R], F32, tag="mm", bufs=3, name="ln_mu")
    for a in range(DT):
        nc.tensor.matmul(mups, onesi, xst[:, a, :], start=(a == 0), stop=(a == DT - 1))
    drain_filler(3)
    sq = tmps.tile([128, DT, R], BF16, tag="sq", bufs=1, name="ln_sqt")
    nc.vector.tensor_mul(sq, xst, xst)
    sqps = psum.tile([128, R], F32, tag="mm", bufs=3, name="ln_sq")
    for a in range(DT):
        nc.tensor.matmul(sqps, onesi, sq[:, a, :], start=(a == 0), stop=(a == DT - 1))
    drain_filler(5)
    musb = tmps.tile([128, R], F32, tag="lns", bufs=2, name="ln_musb")
    nc.vector.tensor_copy(musb, mups)
    mu2 = tmps.tile([128, R], F32, tag="lns", bufs=2, name="ln_mu2")
    nc.vector.tensor_mul(mu2, musb, musb)
    var = tmps.tile([128, R], F32, tag="lns", bufs=2, name="ln_var")
    nc.vector.tensor_sub(var, sqps, mu2)
    sd = tmps.tile([128, R], F32, tag="lns", bufs=2, name="ln_sd")
    nc.scalar.activation(out=sd, in_=var, func=AF.Sqrt, bias=epsln, scale=1.0)
    rstd = tmps.tile([128, R], F32, tag="lns", bufs=2, name="ln_rstd")
    nc.vector.reciprocal_approx_fast(rstd, sd)
    xm = tmps.tile([128, DT, R], F32, tag="lnxm", bufs=1, name="ln_xm")
    nc.vector.tensor_sub(xm, x, _bc(mups, DT))
    t = xm
    nc.vector.tensor_mul(t, xm, _bc(rstd, DT))
    for a in range(DT):
        # bf16/fp8 copy first: it unblocks the next matmul consumer
        nc.scalar.activation(out=out_bf[:, a, :], in_=t[:, a, :], func=AF.Identity,
                             bias=prm[:, bcol + a:bcol + a + 1],
                             scale=prm[:, gcol:gcol + 1])
        if write_f32:
            nc.vector.tensor_scalar(out=x[:, a, :], in0=t[:, a, :],
                                    scalar1=prm[:, gcol:gcol + 1],
                                    scalar2=prm[:, bcol + a:bcol + a + 1],
                                    op0=ALU.mult, op1=ALU.add)


def _build():
    nc = bacc.Bacc("TRN2", target_bir_lowering=False, debug=False)

    # all host-swizzled to [128, K] (partition-major, one 2D DMA each)
    memT_d = nc.dram_tensor("memT8", [128, DT * M], F8, kind="ExternalInput").ap()
    xT0_d = nc.dram_tensor("xT0", [128, DT * R], F32, kind="ExternalInput").ap()
    xT0bf_d = nc.dram_tensor("xT0bf", [128, DT * R], BF16, kind="ExternalInput").ap()
    xT08_d = nc.dram_tensor("xT08", [128, DT * R], F8, kind="ExternalInput").ap()
    thr_d = nc.dram_tensor("thr", [128, R], BF16, kind="ExternalInput").ap()
    ind_d = nc.dram_tensor("ind", [128, MT * 128], BF16, kind="ExternalInput").ap()
    prm_d = nc.dram_tensor("params", [128, P_COLS], F32, kind="ExternalInput").ap()
    WDT = F8 if FFN_FP8 else BF16
    wq_d, wk_d, wv_d, w1_d, w2_d = [], [], [], [], []
    for l in range(L):
        # wq j-major so qproj j=0 can start on a 64KB DMA
        wq_d.append(nc.dram_tensor(f"wq8{l}", [128, DT * D], F8, kind="ExternalInput").ap())
        wk_d.append(nc.dram_tensor(f"wk8{l}", [128, DT * D], F8, kind="ExternalInput").ap())
        wv_d.append(nc.dram_tensor(f"wv8{l}", [128, DT * D], F8, kind="ExternalInput").ap())
        # w1 chunk-major: [(og a) col] with 4 chunks, w2 plain: [(og s) col]
        w1_d.append(nc.dram_tensor(f"w1{l}", [128, 4 * DT * 512], WDT, kind="ExternalInput").ap())
        w2_d.append(nc.dram_tensor(f"w2{l}", [128, FT * D], WDT, kind="ExternalInput").ap())
    ws_d = nc.dram_tensor("ws", [128, 8 * D], BF16, kind="ExternalInput").ap()
    out_d = nc.dram_tensor("outT", [D, R], F32, kind="ExternalOutput").ap()

    with tile.TileContext(nc) as tc:
        with (
            tc.tile_pool(name="sb", bufs=1) as sb,
            tc.tile_pool(name="ps", bufs=1, space="PSUM") as ps,
        ):
            # --- resident inputs; DMA issue order = need order ------------------
            prm = sb.tile([128, P_COLS], F32, tag="prm", name="prm_sb")
            nc.sync.dma_start(out=prm, in_=prm_d[:, :])
            x08 = sb.tile([128, DT, R], F8, tag="x08", name="x08_sb")
            nc.sync.dma_start(out=x08, in_=xT08_d.rearrange("p (a n) -> p a n", a=DT))

            def load_layer_weights(l):
                # wq j-major layout: [128, j, a, 128]
                wq = sb.tile([128, DT, DT, 128], F8, tag="wq", bufs=2, name="wq_sb")
                for j in range(DT):
                    nc.sync.dma_start(
                        out=wq[:, j, :, :],
                        in_=wq_d[l][:, ds(j * DT * 128, DT * 128)].rearrange(
                            "p (a n) -> p a n", a=DT))
                wk = sb.tile([128, DT, D], F8, tag="wk", bufs=2, name="wk_sb")
                wv = sb.tile([128, DT, D], F8, tag="wv", bufs=2, name="wv_sb")
                nc.sync.dma_start(out=wk, in_=wk_d[l].rearrange("p (a n) -> p a n", a=DT))
                nc.sync.dma_start(out=wv, in_=wv_d[l].rearrange("p (a n) -> p a n", a=DT))
                return wq, wk, wv

            layer_w = [load_layer_weights(0)]
            memT = sb.tile([128, DT, M], F8, tag="memT", name="memT_sb")
            for c0, c1 in ((0, 1024), (1024, 2048), (2048, M)):
                for a in range(DT):
                    nc.sync.dma_start(out=memT[:, a, c0:c1],
                                      in_=memT_d[:, a * M + c0:a * M + c1])
            x0bf = sb.tile([128, DT, R], BF16, tag="x0bf", name="x0bf_sb")
            nc.sync.dma_start(out=x0bf, in_=xT0bf_d.rearrange("p (a n) -> p a n", a=DT))
            x = sb.tile([128, DT, R], F32, tag="x", name="x_sb")
            x0 = sb.tile([128, DT, R], F32, tag="x0", name="x0_sb")
            nc.sync.dma_start(out=x0, in_=xT0_d.rearrange("p (a n) -> p a n", a=DT))
            # x starts as a copy of x0 (gpsimd: off the critical engines)
            for a in range(DT):
                nc.gpsimd.tensor_copy(x[:, a, :], x0[:, a, :])
            thr = sb.tile([128, R], BF16, tag="thr", name="thr_sb")
            nc.sync.dma_start(out=thr, in_=thr_d[:, :])
            indb = sb.tile([128, MT, 128], BF16, tag="ind", name="ind_sb")
            nc.sync.dma_start(out=indb, in_=ind_d.rearrange("p (t m) -> p t m", m=128))
            ones8 = sb.tile([128, 2, 128], F8, tag="ones8", name="ones8_sb")
            nc.vector.memset(ones8, 1.0)
            onesi = sb.tile([128, 128], BF16, tag="onesi", name="onesi_sb")
            nc.vector.memset(onesi, 1.0 / D)
            epsln = sb.tile([128, 1], F32, tag="epsln", name="epsln_sb")
            nc.vector.memset(epsln, 1e-5)

            # kproj group: kT8[:, j, c*512:+512] = DoubleRow(wk, memT) + bk
            def emit_kproj_group(l, wk, kT, c, j, tag="mm", tagbufs=3,
                                 cast_on_scalar=False):
                base = 48 * l
                kps = ps.tile([128, 512], F32, tag=tag, bufs=tagbufs, name="k_ps")
                for a2 in range(DT // 2):
                    nc.tensor.matmul(kps, wk[:, ds(2 * a2, 2), ds(j * 128, 128)],
                                     memT[:, ds(2 * a2, 2), ds(c * 512, 512)],
                                     start=(a2 == 0), stop=(a2 == DT // 2 - 1),
                                     perf_mode=DR)
                dst = kT[:, j, ds(c * 512, 512)]
                bias_ap = prm[:, base + P_BK + j:base + P_BK + j + 1]
                if cast_on_scalar:
                    nc.scalar.activation(out=dst, in_=kps, func=AF.Identity,
                                         bias=bias_ap, scale=1.0)
                else:
                    nc.vector.tensor_scalar(out=dst, in0=kps, scalar1=bias_ap,
                                            scalar2=None, op0=ALU.add)

            # filler queue: pending PE work for the next layer, drained into
            # the PE-idle zones (LN chains / FFN) of the current layer
            filler: list = []

            def drain_filler(n):
                for _ in range(min(n, len(filler))):
                    filler.pop(0)()

            # per-layer per-feature bias broadcast tiles (bv, b2), built
            # off the critical path on Scalar
            def make_bias_bcast(col, name):
                bt = sb.tile([128, DT, 1], F32, tag="bt", bufs=2, name=name)
                for j in range(DT):
                    nc.scalar.activation(out=bt[:, j, :], in_=epsln,
                                         func=AF.Identity, scale=0.0,
                                         bias=prm[:, col + j:col + j + 1])
                return bt.broadcast_to([128, DT, R])

            x8cur = x08  # fp8 attention input of the current layer
            vpre_cur = []
            for l in range(L):
                base = 48 * l
                wq, wk, wv = layer_w[l]

                # --- q projection (fp8 DR matmuls, fp8 output) ------------------
                q8 = sb.tile([128, DT, R], F8, tag="q8", bufs=1, name="q_sb")
                for j in range(DT):
                    qps = ps.tile([128, R], F32, tag="mm", bufs=3, name="q_ps")
                    for a2 in range(DT // 2):
                        nc.tensor.matmul(qps, wq[:, j, ds(2 * a2, 2), :],
                                         x8cur[:, ds(2 * a2, 2), :],
                                         start=(a2 == 0), stop=(a2 == DT // 2 - 1),
                                         perf_mode=DR)
                    nc.vector.tensor_scalar(out=q8[:, j, :], in0=qps,
                                            scalar1=prm[:, base + P_BQ + j:base + P_BQ + j + 1],
                                            scalar2=None, op0=ALU.add)

                # --- kT projection ([dout, m] fp8, resident) --------------------
                if l == 0:
                    kT = sb.tile([128, DT, M], F8, tag="kT", name="kT_sb")
                    for c in range(NCHUNK):
                        for j in range(DT):
                            emit_kproj_group(0, wk, kT, c, j)
                else:
                    kT = kT_next  # noqa: F821  (built by layer l-1's filler)
                    drain_filler(len(filler))  # any leftovers

                bvt = make_bias_bcast(base + P_BV, "bvt_sb")

                # --- attention (mt pairs, DoubleRow fp8) ------------------------
                # one big accumulator tile: [128, j, R] = attnT
                bigps = ps.tile([128, DT, R], F32, tag="attn", bufs=1, name="attn_ps")
                zps = ps.tile([128, R], F32, tag="z", bufs=1, name="z_ps")
                for mp in range(MT // 2):
                    if mp < len(vpre_cur):
                        v2 = vpre_cur[mp]  # prefetched during previous layer
                    else:
                        v2 = sb.tile([128, 2, D], F8, tag="vsb", bufs=3, name="v_sb")
                        for i in range(2):
                            mt = 2 * mp + i
                            vps = ps.tile([128, D], F32, tag="mm", bufs=3, name="v_ps")
                            for a2 in range(DT // 2):
                                nc.tensor.matmul(vps, memT[:, ds(2 * a2, 2), ds(mt * 128, 128)],
                                                 wv[:, ds(2 * a2, 2), :],
                                                 start=(a2 == 0), stop=(a2 == DT // 2 - 1),
                                                 perf_mode=DR)
                            if i == 0:
                                nc.vector.tensor_copy(v2[:, i, :], vps)
                            else:
                                nc.scalar.activation(out=v2[:, i, :], in_=vps,
                                                     func=AF.Copy)
                    # scores + mask -> masked exp, per half of the pair
                    e2 = sb.tile([128, 2, R], F8, tag="e", bufs=3, name="e_sb")
                    for i in range(2):
                        mt = 2 * mp + i
                        sps = ps.tile([128, R], F32, tag="mm", bufs=3, name="s_ps")
                        for a2 in range(DT // 2):
                            nc.tensor.matmul(sps, kT[:, ds(2 * a2, 2), ds(mt * 128, 128)],
                                             q8[:, ds(2 * a2, 2), :],
                                             start=(a2 == 0), stop=False,
                                             perf_mode=DR, skip_group_check=True)
                        nc.tensor.matmul(sps, indb[:, mt, :], thr,
                                         start=False, stop=True,
                                         skip_group_check=True)
                        nc.scalar.activation(out=e2[:, i, :], in_=sps,
                                             func=AF.Exp, scale=EXPSC)
                    nc.tensor.matmul(zps, ones8, e2, start=(mp == 0),
                                     stop=(mp == MT // 2 - 1),
                                     perf_mode=DR, skip_group_check=True)
                    for j in range(DT):
                        nc.tensor.matmul(bigps[:, j, :], v2[:, :, ds(j * 128, 128)], e2,
                                         start=(mp == 0), stop=(mp == MT // 2 - 1),
                                         perf_mode=DR, skip_group_check=True)

                # enqueue next layer's kT projection as PE filler work
                vpre_next = []
                if l + 1 < L:
                    layer_w.append(load_layer_weights(l + 1))
                    kT_next = sb.tile([128, DT, M], F8, tag="kT", name="kTn_sb")
                    wk_next = layer_w[l + 1][1]
                    for c in range(NCHUNK):
                        for j in range(DT):
                            tg, tb = ("z", 1) if (c * DT + j) % 2 else ("mm", 3)
                            filler.append(
                                lambda ll=l + 1, c=c, j=j, wkn=wk_next,
                                ktn=kT_next, tg=tg, tb=tb:
                                emit_kproj_group(ll, wkn, ktn, c, j, tg, tb,
                                                 cast_on_scalar=(c + j) % 2 == 0))
                    wv_next = layer_w[l + 1][2]

                    def emit_vpre(mp_, vt):
                        for i in range(2):
                            mt = 2 * mp_ + i
                            tg, tb = ("z", 1) if i else ("mm", 3)
                            vp = ps.tile([128, D], F32, tag=tg, bufs=tb, name="vp_ps")
                            for a2 in range(DT // 2):
                                nc.tensor.matmul(vp, memT[:, ds(2 * a2, 2), ds(mt * 128, 128)],
                                                 wv_next[:, ds(2 * a2, 2), :],
                                                 start=(a2 == 0), stop=(a2 == DT // 2 - 1),
                                                 perf_mode=DR)
                            if i == 0:
                                nc.vector.tensor_copy(vt[:, i, :], vp)
                            else:
                                nc.scalar.activation(out=vt[:, i, :], in_=vp,
                                                     func=AF.Copy)

                    for mp_ in range(8):
                        vt = sb.tile([128, 2, D], F8, tag="vpre", bufs=8,
                                     name="vpre_sb")
                        vpre_next.append(vt)
                        filler.append(lambda mp_=mp_, vt=vt: emit_vpre(mp_, vt))

                # normalize + residual into x, batched:
                # x += bigps * rz + bvt   (1/32 of the v-scale inside rz)
                zt = sb.tile([128, R], F32, tag="at", bufs=2, name="zt_sb")
                nc.scalar.activation(out=zt, in_=zps, func=AF.Copy, bias=1e-9,
                                     scale=float(W8))
                rz = sb.tile([128, R], F32, tag="rz", bufs=1, name="rz_sb")
                nc.vector.reciprocal_approx_fast(rz, zt)
                at = sb.tile([128, DT, R], F32, tag="at2", bufs=1, name="at_sb")
                nc.vector.tensor_mul(at, bigps, _bc(rz, DT))
                at2 = sb.tile([128, DT, R], F32, tag="at3", bufs=1, name="at2_sb")
                nc.vector.tensor_add(at2, at, bvt)
                nc.vector.tensor_add(x, x, at2)
                drain_filler(4)

                # last layer: the gate's x0 half becomes filler work
                if l == L - 1:
                    ws0c = sb.tile([128, DT, 512], BF16, tag="wsc", bufs=2,
                                   name="ws0c_sb")
                    nc.sync.dma_start(
                        out=ws0c,
                        in_=ws_d[:, 0:DT * 512].rearrange("p (s n) -> p s n", s=DT))
                    gstash = sb.tile([128, DT, R], BF16, tag="gstash", bufs=1,
                                     name="gstash_sb")

                    def emit_gate_x0(j):
                        gxp = ps.tile([128, R], F32, tag="z", bufs=1, name="gx_ps")
                        for c in range(DT):
                            nc.tensor.matmul(gxp, ws0c[:, c, ds(j * 128, 128)],
                                             x0bf[:, c, :],
                                             start=(c == 0), stop=(c == DT - 1))
                        nc.vector.tensor_scalar(out=gstash[:, j, :], in0=gxp,
                                                scalar1=prm[:, P_BS + j:P_BS + j + 1],
                                                scalar2=None, op0=ALU.add)

                    for j in range(DT):
                        filler.append(lambda j=j: emit_gate_x0(j))

                # LN1 (in place); fp8 copy feeds the (fp8) FFN1
                xln8 = sb.tile([128, DT, R], F8 if FFN_FP8 else BF16,
                               tag="xbf", bufs=2, name="xln8_sb")
                _layernorm(nc, ps, sb, x, prm, base + P_G1, base + P_BE1, xln8,
                           onesi, epsln, drain_filler)

                b2t = make_bias_bcast(base + P_B2, "b2t_sb")

                # FFN1 -> FFN2 fused over the 4D dim (fp8 DR)
                f2big = ps.tile([128, DT, R], F32, tag="attn", bufs=1, name="f2_ps")

                w1c, w2c = {}, {}

                def load_ffn_chunk(og):
                    w1c[og] = sb.tile([128, DT, 512], WDT, tag="wc", bufs=4,
                                      name="w1c_sb")
                    nc.sync.dma_start(
                        out=w1c[og],
                        in_=w1_d[l][:, ds(og * DT * 512, DT * 512)].rearrange(
                            "p (a n) -> p a n", a=DT))
                    w2c[og] = sb.tile([128, DT, 512], WDT, tag="wc", bufs=4,
                                      name="w2c_sb")
                    nc.sync.dma_start(
                        out=w2c[og],
                        in_=w2_d[l][:, ds(og * DT * D, DT * D)].rearrange(
                            "p (s n) -> p s n", s=DT))

                load_ffn_chunk(0)

                def emit_f2(h2, op):
                    # h2: [128, 2, R] fp8 pair of FFN1 outputs (o=2*op, 2*op+1)
                    o0 = 2 * op
                    for j in range(DT):
                        nc.tensor.matmul(
                            f2big[:, j, :],
                            w2c[o0 // 4][:, ds(o0 % 4, 2), ds(j * 128, 128)], h2,
                            start=(op == 0), stop=(op == FT // 2 - 1),
                            perf_mode=DR, skip_group_check=True)

                hq = []
                h2 = None
                for o in range(FT):
                    fps = ps.tile([128, R], F32, tag="mm", bufs=3, name="f1_ps")
                    for a2 in range(DT // 2):
                        nc.tensor.matmul(fps,
                                         w1c[o // 4][:, ds(2 * a2, 2), ds((o % 4) * 128, 128)],
                                         xln8[:, ds(2 * a2, 2), :],
                                         start=(a2 == 0), stop=(a2 == DT // 2 - 1),
                                         perf_mode=DR)
                    if o % 2 == 0:
                        h2 = sb.tile([128, 2, R], F8, tag="h", bufs=4, name="h_sb")
                    nc.scalar.activation(out=h2[:, o % 2, :], in_=fps, func=AF.Gelu,
                                         bias=prm[:, base + P_B1 + o:base + P_B1 + o + 1],
                                         scale=(1.0 / W8 if FFN_FP8 else 1.0))
                    if o % 2 == 1:
                        hq.append((h2, o // 2))
                        if len(hq) > 1:
                            emit_f2(*hq.pop(0))
                    drain_filler(1)
                    if o % 4 == 3 and o // 4 + 1 < 4:
                        load_ffn_chunk(o // 4 + 1)
                for h_o in hq:
                    emit_f2(*h_o)
                # x += f2big/32 + b2, batched (scale+bias fused in one stt)
                f2b = sb.tile([128, DT, R], F32, tag="at3", bufs=1, name="f2b_sb")
                nc.vector.scalar_tensor_tensor(
                    out=f2b, in0=f2big, scalar=(1.0 / W8 if FFN_FP8 else 1.0),
                    in1=b2t, op0=ALU.mult, op1=ALU.add)
                nc.vector.tensor_add(x, x, f2b)

                if l == L - 1:
                    ws1c = sb.tile([128, DT, 512], BF16, tag="wsc", bufs=2,
                                   name="ws1c_sb")
                    nc.sync.dma_start(
                        out=ws1c,
                        in_=ws_d[:, DT * 512:8 * 512].rearrange("p (s n) -> p s n", s=DT))

                if l + 1 < L:
                    # LN2 (in place) + bf16 copy for next layer
                    xbf = sb.tile([128, DT, R], BF16, tag="xbf", bufs=2, name="xbf_sb")
                    _layernorm(nc, ps, sb, x, prm, base + P_G2, base + P_BE2, xbf,
                               onesi, epsln, drain_filler)
                    # fp8 copy for next layer's qproj
                    x8n = sb.tile([128, DT, R], F8, tag="x8n", bufs=1, name="x8n_sb")
                    nc.vector.tensor_copy(x8n, xbf)
                    x8cur = x8n
                else:
                    # final LN2 + gate + output, split in row halves so the
                    # second half's PE work overlaps the first half's DVE/
                    # Scalar chain, and the out DMA starts at half time.
                    # out = xbf + g*(x0 - xbf); item-0 rows fixed on host.
                    RH = R // 2
                    xbf = sb.tile([128, DT, R], BF16, tag="xbf", bufs=2,
                                  name="xbf_sb")
                    gps = ps.tile([128, DT, R], F32, tag="attn", bufs=1,
                                  name="ga_ps")
                    for rh in range(2):
                        rsl = ds(rh * RH, RH)
                        xst = sb.tile([128, DT, RH], BF16, tag="xst2", bufs=2,
                                      name="fl_xst")
                        nc.vector.tensor_copy(xst, x[:, :, rsl])
                        mups = ps.tile([128, RH], F32, tag="mm", bufs=3, name="fl_mu")
                        for a in range(DT):
                            nc.tensor.matmul(mups, onesi, xst[:, a, :],
                                             start=(a == 0), stop=(a == DT - 1))
                        sq = sb.tile([128, DT, RH], BF16, tag="sq2", bufs=2,
                                     name="fl_sq")
                        nc.vector.tensor_mul(sq, xst, xst)
                        sqps = ps.tile([128, RH], F32, tag="mm", bufs=3, name="fl_sqp")
                        for a in range(DT):
                            nc.tensor.matmul(sqps, onesi, sq[:, a, :],
                                             start=(a == 0), stop=(a == DT - 1))
                        musb = sb.tile([128, RH], F32, tag="lns2", bufs=2, name="fl_musb")
                        nc.vector.tensor_copy(musb, mups)
                        mu2 = sb.tile([128, RH], F32, tag="lns2", bufs=2, name="fl_mu2")
                        nc.vector.tensor_mul(mu2, musb, musb)
                        var = sb.tile([128, RH], F32, tag="lns2", bufs=2, name="fl_var")
                        nc.vector.tensor_sub(var, sqps, mu2)
                        sd = sb.tile([128, RH], F32, tag="lns2", bufs=2, name="fl_sd")
                        nc.scalar.activation(out=sd, in_=var, func=AF.Sqrt,
                                             bias=epsln, scale=1.0)
                        rstd = sb.tile([128, RH], F32, tag="lns2", bufs=2, name="fl_rstd")
                        nc.vector.reciprocal_approx_fast(rstd, sd)
                        xm = sb.tile([128, DT, RH], F32, tag="lnxm2", bufs=2, name="fl_xm")
                        nc.vector.tensor_sub(xm, x[:, :, rsl], _bc(mups, DT, RH))
                        t = xm
                        nc.vector.tensor_mul(t, xm, _bc(rstd, DT, RH))
                        for a in range(DT):
                            nc.scalar.activation(
                                out=xbf[:, a, rsl], in_=t[:, a, :], func=AF.Identity,
                                bias=prm[:, base + P_BE2 + a:base + P_BE2 + a + 1],
                                scale=prm[:, base + P_G2:base + P_G2 + 1])
                        for j in range(DT):
                            for c in range(DT):
                                nc.tensor.matmul(gps[:, j, rsl],
                                                 ws1c[:, c, ds(j * 128, 128)],
                                                 xbf[:, c, rsl],
                                                 start=(c == 0), stop=(c == DT - 1),
                                                 skip_group_check=True)
                        dxh = sb.tile([128, DT, RH], F32, tag="dxh", bufs=2,
                                      name="fl_dx")
                        nc.vector.tensor_sub(dxh, x0[:, :, rsl], xbf[:, :, rsl])
                        tgh = sb.tile([128, DT, RH], F32, tag="gt", bufs=3,
                                      name="fl_tg")
                        nc.vector.tensor_add(tgh, gps[:, :, rsl], gstash[:, :, rsl])
                        gh = sb.tile([128, DT, RH], F32, tag="gt", bufs=3,
                                     name="fl_g")
                        nc.scalar.activation(out=gh, in_=tgh, func=AF.Sigmoid)
                        m2h = sb.tile([128, DT, RH], F32, tag="gt", bufs=3,
                                      name="fl_m2")
                        nc.vector.tensor_mul(m2h, gh, dxh)
                        ovh = sb.tile([128, DT, RH], F32, tag="gt", bufs=3,
                                      name="fl_ov")
                        nc.vector.tensor_add(ovh, xbf[:, :, rsl], m2h)
                        for j in range(DT):
                            nc.sync.dma_start(
                                out=out_d[j * 128:(j + 1) * 128,
                                          rh * RH:(rh + 1) * RH],
                                in_=ovh[:, j, :])
                vpre_cur = vpre_next

    nc.compile()
    return nc


_NC = None


def _get_nc():
    global _NC
    if _NC is None:
        _NC = _build()
    return _NC


def _sw(x, p=128):
    """[A*p, N] -> [p, A*N] partition-major swizzle (row pp = concat_a x[a*p+pp])."""
    a = x.shape[0] // p
    return np.ascontiguousarray(
        x.reshape(a, p, -1).transpose(1, 0, 2).reshape(p, -1))


def _make_in_maps(inputs):
    cog = np.asarray(inputs["cognition_features"], np.float32)
    flat = cog.reshape(M, D)
    cogT = np.ascontiguousarray(flat.T)          # [D, M] fp32
    wdt = F8NP if FFN_FP8 else BF
    wsc = W8 if FFN_FP8 else 1.0

    common = {"memT8": _sw(cogT).astype(F8NP)}
    for l in range(L):
        # wq j-major: [p, j, a, 128] from (32*Wq).T [D, D]
        wqt = np.asarray(inputs["Wq"][l], np.float32).T * W8
        common[f"wq8{l}"] = np.ascontiguousarray(
            wqt.reshape(DT, 128, DT, 128).transpose(1, 2, 0, 3).reshape(128, -1)
        ).astype(F8NP)
        common[f"wk8{l}"] = _sw(
            np.asarray(inputs["Wk"][l], np.float32).T * W8).astype(F8NP)
        common[f"wv8{l}"] = _sw(
            np.asarray(inputs["Wv"][l], np.float32).T * W8).astype(F8NP)
        # w1 chunk-major: [p, (og a) col] from (wsc*W1).T [D, 4D]
        w1t = np.asarray(inputs["W1"][l], np.float32).T * wsc
        common[f"w1{l}"] = np.ascontiguousarray(
            w1t.reshape(DT, 128, 4, 512).transpose(1, 2, 0, 3).reshape(128, -1)
        ).astype(wdt)
        common[f"w2{l}"] = _sw(
            np.asarray(inputs["W2"][l], np.float32).T * wsc).astype(wdt)
    common["ws"] = _sw(np.asarray(inputs["Ws"], np.float32).T).astype(BF)

    prm = np.zeros((128, P_COLS), np.float32)

    def put(col, vec):
        v = np.asarray(vec, np.float32).reshape(-1, 128)
        for j in range(v.shape[0]):
            prm[:, col + j] = v[j]

    for l in range(L):
        base = 48 * l
        put(base + P_BQ, np.asarray(inputs["bq"][l], np.float32) * W8)
        put(base + P_BK, np.asarray(inputs["bk"][l], np.float32) * W8)
        put(base + P_BV, inputs["bv"][l])
        put(base + P_G1, inputs["ln1_g"][l])
        put(base + P_BE1, inputs["ln1_b"][l])
        put(base + P_B1, inputs["b1"][l])
        put(base + P_B2, inputs["b2"][l])
        put(base + P_G2, inputs["ln2_g"][l])
        put(base + P_BE2, inputs["ln2_b"][l])
    put(P_BS, inputs["bs"])
    common["params"] = prm

    # ind[theta, mt*128+p] = 1 iff theta == item(mt*128+p)  (mask rank factor)
    item_of_m = np.arange(M) // T
    ind = np.zeros((128, MT * 128), np.float32)
    ind[item_of_m, np.arange(M)] = 1.0
    common["ind"] = ind.astype(BF)

    in_maps = []
    for d in range(NCORES):
        rows = slice(d * R, (d + 1) * R)
        b_of_r = np.arange(d * R, (d + 1) * R) // T
        im = dict(common)
        xt = np.ascontiguousarray(cogT[:, rows])
        im["xT0"] = _sw(xt)
        im["xT0bf"] = _sw(xt.astype(BF))
        im["xT08"] = _sw(xt.astype(F8NP))
        # thr[theta, r] = MASKV iff theta >= item(r) (else 0)
        th = np.where(np.arange(128)[:, None] >= b_of_r[None, :], MASKV, 0.0)
        im["thr"] = th.astype(BF)
        in_maps.append(im)
    return in_maps


def _run(in_maps, trace=False):
    nc = _get_nc()
    return run_bass_kernel_spmd(nc, in_maps, list(range(NCORES)), trace=trace)


def kernel(**inputs):
    in_maps = _make_in_maps(inputs)
    res = _run(in_maps)
    outT = np.empty((M, D), np.float32)
    for d in range(NCORES):
        outT[d * R:(d + 1) * R, :] = res.results[d]["outT"].T
    out = outT.reshape(B, T, D)
    # item 0 attends over an empty bank: out == input there
    out[0] = np.asarray(inputs["cognition_features"], np.float32)[0]
    return out


if __name__ == "__main__":
    _build()
    print("build ok")
